# revision 1
# baseline (speedup 1.0000x reference)
"""DinoV2 detection loss on 8 Trainium2 NeuronCores (Bass/Tile).

Reference computation (per batch sample b; B=128, Q=2048, C=365, T=50):
  dist[q, t] = sum_d |pred_boxes[b,q,d] - target_boxes[b,t,d]|
  closest[t] = argmin_q dist[q, t]
  class_targets = scatter(zeros(Q), closest, labels)     (last write wins)
  loss_ce  = weighted CE over all Q rows (background cls 0 weight 0.1)
  loss_bbox = mean_t,d |pred_boxes[closest[t]] - target_boxes[t]|
  out = mean_b(2*loss_ce + 5*loss_bbox)

Sharding: data-parallel over B; each core handles 16 samples and emits
16 per-sample losses; host averages 128 values.

Per-core device algorithm (samples processed in 8 pairs of 2, laid out on
100 partitions = 2 x 50 targets):
  - Main CE pass over a host-transposed bf16 copy of the logits
    [sample, class, query]: ACT exponentiates whole class-chunks
    ([128, 2048] per op, bf16 out), PE reduces classes via a ones-vector
    matmul (f32 PSUM), ACT takes Ln directly from PSUM -> row LSE.
    S_b = sum_q (LSE - logit[...,0]) via one DVE subtract + reduce.
  - Distances: PE matmul trick gives diff[t,q] = pb[q,d] - tb[t,d]
    (contraction over indicator/value rows; boxes split hi/lo into two
    bf16 components for near-fp32 accuracy, K=6), DVE reduces |diff|
    over d, reduce-min + max_index give (min dist, argmin).
  - Matched corrections: indirect-DMA gather of the 50 matched logit
    rows per sample from the row-major f32 logits, exp+accum for their
    LSE, one-hot dot for the target-class logit, duplicate-match
    resolution via an equality matrix against the transposed index
    vector (last write wins).
"""

import numpy as np

B, Q, C, T = 128, 2048, 365, 50
NCORES = 8
NLOC = B // NCORES          # 16 samples per core
NPAIR = NLOC // 2           # 8 pairs
P2 = 2 * T                  # 100 partitions per pair tile
QCH = 256                   # dist matmul free-dim chunk
NQC = Q // QCH              # 8
QRS = 512                   # row-sum matmul free-dim chunk
NRS = Q // QRS              # 4
CCH = [(0, 128), (128, 128), (256, 109)]  # class chunks (start, rows)
W_BG = float(np.float32(0.1))
DEN0 = float(np.float32(0.1) * 2048)   # background weight sum

_CACHE = {}


def _build_nc():
    import concourse.bacc as bacc
    import concourse.bass as bass
    import concourse.mybir as mybir
    import concourse.tile as tile

    f32 = mybir.dt.float32
    bf16 = mybir.dt.bfloat16
    Alu = mybir.AluOpType
    Act = mybir.ActivationFunctionType
    Ax = mybir.AxisListType

    nc = bacc.Bacc("TRN2", target_bir_lowering=False, debug=False)

    # row-major f32 logits: only read by the matched-row indirect gather
    logits = nc.dram_tensor("logits", [NLOC * Q, C], f32, kind="ExternalInput")
    # transposed bf16 logits for the bulk CE pass, repacked as
    # [sample, class-chunk, q-half, class-in-chunk, q-in-half]: each
    # (sample, chunk) block is one contiguous 512KB region whose DMA
    # partition stride is 2KB -- both properties are needed for the
    # descriptor splitter to spread the transfer across all 16 SDMA
    # engines. Classes padded 365->384 with -30 (exp ~ 0).
    logits_q = nc.dram_tensor(
        "logits_q", [NLOC, 3, 2, 128, Q // 2], bf16, kind="ExternalInput"
    )
    # hi/lo bf16 split of the box-diff matmul operands (K=6 contraction)
    mmrhs = nc.dram_tensor("mmrhs", [6 * NPAIR, 4, Q], bf16, kind="ExternalInput")
    mmlhs = nc.dram_tensor("mmlhs", [6 * NPAIR, 4, P2], bf16, kind="ExternalInput")
    labels = nc.dram_tensor("labels", [NLOC, T], f32, kind="ExternalInput")
    iota_c = nc.dram_tensor("iota_c", [128, C], f32, kind="ExternalInput")
    ident = nc.dram_tensor("ident", [128, 128], f32, kind="ExternalInput")
    trimask = nc.dram_tensor("trimask", [P2, P2], f32, kind="ExternalInput")
    halfoff = nc.dram_tensor("halfoff", [P2, 1], f32, kind="ExternalInput")
    ones16 = nc.dram_tensor("ones16", [128, 16], bf16, kind="ExternalInput")
    blockhalf = nc.dram_tensor("blockhalf", [P2, 2], f32, kind="ExternalInput")
    loss16 = nc.dram_tensor("loss16", [2, NPAIR], f32, kind="ExternalOutput")

    with tile.TileContext(nc) as tc:
        with (
            tc.tile_pool(name="const", bufs=1) as cpool,
            tc.tile_pool(name="logits", bufs=6) as lpool,
            tc.tile_pool(name="expbf", bufs=5) as epool,
            tc.tile_pool(name="scr", bufs=2) as spool,
            tc.tile_pool(name="acc", bufs=1) as apool,
            tc.tile_pool(name="pair", bufs=3) as ppool,
            tc.tile_pool(name="dram", bufs=1, space="DRAM") as dpool,
            tc.tile_pool(name="psd", bufs=2, space="PSUM") as psd,
            tc.tile_pool(name="psr", bufs=3, space="PSUM") as psr,
            tc.tile_pool(name="psh", bufs=1, space="PSUM") as psh,
        ):
            # ---- constants into SBUF (early: needed by pairs / samples) ----
            ones_sb = cpool.tile([128, 16], bf16, tag="ones")
            nc.sync.dma_start(out=ones_sb[:], in_=ones16.ap())
            ident_sb = cpool.tile([128, 128], f32, tag="ident")
            nc.sync.dma_start(out=ident_sb[:], in_=ident.ap())
            tri_sb = cpool.tile([P2, P2], f32, tag="tri")
            nc.sync.dma_start(out=tri_sb[:], in_=trimask.ap())
            hoff_sb = cpool.tile([P2, 1], f32, tag="hoff")
            nc.sync.dma_start(out=hoff_sb[:], in_=halfoff.ap())
            # labels -> [100, 8]: partition (h*50+t), col p holds labels[2p+h, t]
            lab_sb = cpool.tile([P2, NPAIR], f32, tag="lab")
            lab_src = bass.AP(
                tensor=labels, offset=0, ap=[[T, 2], [1, T], [2 * T, NPAIR]]
            )
            nc.sync.dma_start(out=lab_sb[:], in_=lab_src)
            # cold constants (needed later) are DMA'd after sample 0
            iota_sb = cpool.tile([128, C], f32, tag="iota")
            bh_sb = cpool.tile([P2, 2], f32, tag="bh")

            # ---- accumulators ----
            sumexp_all = apool.tile([NLOC, Q], f32, tag="sumexp")
            lse_all = apool.tile([NLOC, Q], f32, tag="lse")
            l0_all = apool.tile([NLOC, Q], f32, tag="l0")
            mind_all = apool.tile([P2, NPAIR], f32, tag="mind")
            mask_all = apool.tile([P2, NPAIR], f32, tag="mask")
            sume_all = apool.tile([P2, NPAIR], f32, tag="sume")
            ly_all = apool.tile([P2, NPAIR], f32, tag="ly")
            l0m_all = apool.tile([P2, NPAIR], f32, tag="l0m")

            # l0 (class-0 logits) for all rows, bf16 -> f32 cast during DMA
            for qh in range(2):
                nc.gpsimd.dma_start(
                    out=l0_all[:, qh * (Q // 2) : (qh + 1) * (Q // 2)],
                    in_=logits_q.ap()[:, 0, qh, 0, :],
                )

            def emit_sample(s):
                # cc-major matmul emission: each exp chunk immediately feeds
                # the first three (qh,lh) accumulation groups so PE never
                # waits for the whole sample's exps; group 3 trails.
                groups = [(0, 0), (0, 1), (1, 0), (1, 1)]
                exps = []
                pss = []
                for _g in range(3):
                    ps_g = psr.tile([16, QRS], f32, tag="psr", name=f"ps_g{_g}")
                    pss.append(ps_g)
                for cc in range(3):
                    ch = lpool.tile([128, 2, Q // 2], bf16, tag="chunk")
                    nc.sync.dma_start(
                        out=ch[:],
                        in_=logits_q.ap()[s, cc, :, :, :].rearrange(
                            "qh c l -> c qh l"
                        ),
                    )
                    eb = epool.tile([128, 2, Q // 2], bf16, tag="expbf")
                    nc.scalar.activation(eb[:], ch[:], Act.Exp)
                    exps.append(eb)
                    for g in range(3):
                        qh, lh = groups[g]
                        nc.tensor.matmul(
                            out=pss[g][:],
                            lhsT=ones_sb[:],
                            rhs=eb[:, qh, lh * QRS : (lh + 1) * QRS],
                            start=(cc == 0),
                            stop=(cc == 2),
                        )
                se_s = spool.tile([1, Q], f32, tag="se_s")
                for g in range(3):
                    qh, lh = groups[g]
                    q0 = qh * (Q // 2) + lh * QRS
                    nc.scalar.copy(se_s[:, q0 : q0 + QRS], pss[g][0:1, :])
                ps3 = psr.tile([16, QRS], f32, tag="psr")
                qh, lh = groups[3]
                for cc, eb in enumerate(exps):
                    nc.tensor.matmul(
                        out=ps3[:],
                        lhsT=ones_sb[:],
                        rhs=eb[:, qh, lh * QRS : (lh + 1) * QRS],
                        start=(cc == 0),
                        stop=(cc == 2),
                    )
                q0 = qh * (Q // 2) + lh * QRS
                nc.scalar.copy(se_s[:, q0 : q0 + QRS], ps3[0:1, :])
                nc.gpsimd.dma_start(out=sumexp_all[s : s + 1, :], in_=se_s[:])

            def emit_pair(p):
                sl6 = slice(6 * p, 6 * p + 6)
                rhs_t = ppool.tile([6, 4, Q], bf16, tag="rhs_t")
                nc.sync.dma_start(out=rhs_t[:], in_=mmrhs.ap()[sl6, :, :])
                lhs_t = ppool.tile([6, 4, P2], bf16, tag="lhs_t")
                nc.sync.dma_start(out=lhs_t[:], in_=mmlhs.ap()[sl6, :, :])
                dist = ppool.tile([P2, Q], f32, tag="dist")
                for qc in range(NQC):
                    ps = psd.tile([P2, 4, QCH], f32, tag="psd")
                    for d in range(4):
                        nc.tensor.matmul(
                            out=ps[:, d, :],
                            lhsT=lhs_t[:, d, :],
                            rhs=rhs_t[:, d, qc * QCH : (qc + 1) * QCH],
                            start=True,
                            stop=True,
                        )
                    nc.vector.tensor_reduce(
                        out=dist[:, qc * QCH : (qc + 1) * QCH],
                        in_=ps[:].rearrange("p d q -> p q d"),
                        axis=Ax.X,
                        op=Alu.add,
                        apply_absolute_value=True,
                    )
                nc.vector.tensor_reduce(
                    out=mind_all[:, p : p + 1], in_=dist[:], axis=Ax.X, op=Alu.min
                )
                mind8 = ppool.tile([P2, 8], f32, tag="mind8")
                nc.scalar.copy(
                    mind8[:], mind_all[:, p : p + 1].to_broadcast([P2, 8])
                )
                idxu = ppool.tile([P2, 8], mybir.dt.uint32, tag="idxu")
                nc.vector.max_index(out=idxu[:], in_max=mind8[:], in_values=dist[:])
                idxf = ppool.tile([P2, 1], f32, tag="idxf")
                nc.vector.tensor_copy(out=idxf[:], in_=idxu[:, 0:1])
                rowf = ppool.tile([P2, 1], f32, tag="rowf")
                nc.vector.tensor_scalar(
                    rowf[:],
                    idxf[:],
                    hoff_sb[:],
                    float(p * 2 * Q),
                    op0=Alu.add,
                    op1=Alu.add,
                )
                rowi = ppool.tile([P2, 1], mybir.dt.int32, tag="rowi")
                nc.vector.tensor_copy(out=rowi[:], in_=rowf[:])

                # duplicate detection: E[t,t'] = (row[t]==row[t']); count later dups
                idxT_ps = psh.tile([P2, P2], f32, tag="share")
                nc.tensor.transpose(
                    out=idxT_ps[:],
                    in_=rowf[:].to_broadcast([P2, P2]),
                    identity=ident_sb[:P2, :P2],
                )
                idxT = ppool.tile([P2, P2], f32, tag="idxTsb")
                nc.scalar.copy(idxT[:], idxT_ps[:])
                eqm = ppool.tile([P2, P2], f32, tag="eqm")
                nc.vector.tensor_tensor(
                    out=eqm[:],
                    in0=rowf[:].to_broadcast([P2, P2]),
                    in1=idxT[:],
                    op=Alu.is_equal,
                )
                dummy100 = ppool.tile([P2, P2], f32, tag="dummy100")
                cnt = ppool.tile([P2, 1], f32, tag="cnt")
                nc.vector.scalar_tensor_tensor(
                    out=dummy100[:],
                    in0=eqm[:],
                    scalar=1.0,
                    in1=tri_sb[:],
                    op0=Alu.mult,
                    op1=Alu.mult,
                    accum_out=cnt[:],
                )
                nc.vector.tensor_scalar(
                    mask_all[:, p : p + 1],
                    cnt[:],
                    0.0,
                    None,
                    op0=Alu.is_equal,
                )

                # gather matched logit rows (row-major f32 copy)
                rows_sb = ppool.tile([P2, C], f32, tag="rows")
                nc.gpsimd.indirect_dma_start(
                    out=rows_sb[:],
                    out_offset=None,
                    in_=logits.ap(),
                    in_offset=bass.IndirectOffsetOnAxis(ap=rowi[:, 0:1], axis=0),
                )
                return rows_sb

            def emit_matched(p, rows_sb):
                scr2 = spool.tile([P2, C], f32, tag="expdump")
                nc.scalar.activation(
                    scr2[:],
                    rows_sb[:],
                    Act.Exp,
                    accum_out=sume_all[:, p : p + 1],
                )
                oh = ppool.tile([P2, C], f32, tag="oh")
                nc.vector.tensor_scalar(
                    oh[:],
                    iota_sb[:P2, :],
                    lab_sb[:, p : p + 1],
                    None,
                    op0=Alu.is_equal,
                )
                dummyC = ppool.tile([P2, C], f32, tag="dummyC")
                nc.vector.scalar_tensor_tensor(
                    out=dummyC[:],
                    in0=rows_sb[:],
                    scalar=1.0,
                    in1=oh[:],
                    op0=Alu.mult,
                    op1=Alu.mult,
                    accum_out=ly_all[:, p : p + 1],
                )
                nc.vector.tensor_copy(
                    out=l0m_all[:, p : p + 1], in_=rows_sb[:, 0:1]
                )

            # emit main pass with pair work interleaved: pairs run ~2 samples
            # ahead of their own samples (they only need the box inputs);
            # matched-row work trails its pair by ~4 samples so the indirect
            # gather is long complete when ACT reaches it.
            rows_tiles = {}
            for s in range(NLOC):
                emit_sample(s)
                if s == 0:
                    rows_tiles[0] = emit_pair(0)
                    rows_tiles[1] = emit_pair(1)
                    nc.gpsimd.dma_start(out=iota_sb[:], in_=iota_c.ap())
                    nc.gpsimd.dma_start(out=bh_sb[:], in_=blockhalf.ap())
                if s % 2 == 1:
                    p_next = s // 2 + 2
                    if p_next < NPAIR:
                        rows_tiles[p_next] = emit_pair(p_next)
                    m = s // 2
                    if m < NPAIR - 1:
                        emit_matched(m, rows_tiles[m])
                    if s == 13:
                        emit_matched(NPAIR - 1, rows_tiles[NPAIR - 1])

            # ---- main CE reduction: S_b = sum_q (LSE - l0) ----
            nc.scalar.activation(lse_all[:], sumexp_all[:], Act.Ln)
            diff = apool.tile([NLOC, Q], f32, tag="diff")
            nc.vector.tensor_sub(diff[:], lse_all[:], l0_all[:])
            s16 = apool.tile([NLOC, 1], f32, tag="s16")
            nc.vector.tensor_reduce(
                out=s16[:], in_=diff[:], axis=Ax.X, op=Alu.add
            )
            # [16,1] -> [2,8] via DRAM bounce: s = 2p + h
            s16d = dpool.tile([1, NLOC], f32, tag="s16d")
            nc.gpsimd.dma_start(out=s16d[:], in_=s16[:])
            s2 = apool.tile([2, NPAIR], f32, tag="s2")
            nc.gpsimd.dma_start(
                out=s2[:], in_=s16d[:].rearrange("o (pp h) -> o h pp", h=2)
            )

            # ---- matched-term assembly ----
            lsem = apool.tile([P2, NPAIR], f32, tag="lsem")
            nc.scalar.activation(lsem[:], sume_all[:], Act.Ln)
            wy = apool.tile([P2, NPAIR], f32, tag="wy")
            # wy = 1 - 0.9*(label==0)
            nc.vector.tensor_scalar(
                wy[:], lab_sb[:], 0.0, None, op0=Alu.is_equal
            )
            nc.vector.tensor_scalar(
                wy[:], wy[:], -(1.0 - W_BG), 1.0, op0=Alu.mult, op1=Alu.add
            )
            nllm = apool.tile([P2, NPAIR], f32, tag="nllm")
            nc.vector.tensor_sub(nllm[:], lsem[:], ly_all[:])
            stack3 = apool.tile([P2, 3 * NPAIR], f32, tag="stack3")
            corr = stack3[:, 0:NPAIR]
            nc.vector.tensor_mul(corr, wy[:], nllm[:])
            t2 = apool.tile([P2, NPAIR], f32, tag="t2")
            nc.vector.tensor_scalar(
                t2[:], lsem[:], -W_BG, None, op0=Alu.mult
            )
            nc.vector.tensor_add(corr, corr, t2[:])
            nc.vector.tensor_scalar(
                t2[:], l0m_all[:], W_BG, None, op0=Alu.mult
            )
            nc.vector.tensor_add(corr, corr, t2[:])
            nc.vector.tensor_mul(corr, corr, mask_all[:])
            wadd = stack3[:, NPAIR : 2 * NPAIR]
            nc.vector.tensor_scalar(
                wadd, wy[:], -W_BG, None, op0=Alu.add
            )
            nc.vector.tensor_mul(wadd, wadd, mask_all[:])
            nc.vector.tensor_copy(out=stack3[:, 2 * NPAIR :], in_=mind_all[:])

            ps_c = psh.tile([2, 3 * NPAIR], f32, tag="share")
            nc.tensor.matmul(
                out=ps_c[:], lhsT=bh_sb[:], rhs=stack3[:], start=True, stop=True
            )

            # ---- final per-sample combine on [2, 8] ----
            num = apool.tile([2, NPAIR], f32, tag="num")
            nc.vector.tensor_scalar(num[:], s2[:], W_BG, None, op0=Alu.mult)
            nc.vector.tensor_add(num[:], num[:], ps_c[:, 0:NPAIR])
            den = apool.tile([2, NPAIR], f32, tag="den")
            nc.vector.tensor_scalar(
                den[:], ps_c[:, NPAIR : 2 * NPAIR], DEN0, None, op0=Alu.add
            )
            rden = apool.tile([2, NPAIR], f32, tag="rden")
            nc.vector.reciprocal(rden[:], den[:])
            lce = apool.tile([2, NPAIR], f32, tag="lce")
            nc.vector.tensor_mul(lce[:], num[:], rden[:])
            nc.vector.tensor_scalar(lce[:], lce[:], 2.0, None, op0=Alu.mult)
            bbox = apool.tile([2, NPAIR], f32, tag="bbox")
            nc.vector.tensor_scalar(
                bbox[:], ps_c[:, 2 * NPAIR :], 5.0 / (T * 4), None, op0=Alu.mult
            )
            out_sb = apool.tile([2, NPAIR], f32, tag="out")
            nc.vector.tensor_add(out_sb[:], lce[:], bbox[:])
            nc.sync.dma_start(out=loss16.ap(), in_=out_sb[:])

    nc.compile()
    return nc


def get_nc():
    if "nc" not in _CACHE:
        _CACHE["nc"] = _build_nc()
    return _CACHE["nc"]


def _consts():
    import ml_dtypes

    iota = np.broadcast_to(np.arange(C, dtype=np.float32), (128, C)).copy()
    identm = np.eye(128, dtype=np.float32)
    tt, tp = np.meshgrid(np.arange(P2), np.arange(P2), indexing="ij")
    trimask = (tp > tt).astype(np.float32)
    halfoff = ((np.arange(P2) >= T) * Q).astype(np.float32)[:, None]
    ones16 = np.ones((128, 16), ml_dtypes.bfloat16)
    blockhalf = np.zeros((P2, 2), np.float32)
    blockhalf[:T, 0] = 1.0
    blockhalf[T:, 1] = 1.0
    return {
        "iota_c": iota,
        "ident": identm,
        "trimask": trimask,
        "halfoff": halfoff,
        "ones16": ones16,
        "blockhalf": blockhalf,
    }


def _bf16_split(x):
    import ml_dtypes

    hi = x.astype(ml_dtypes.bfloat16)
    lo = (x - hi.astype(np.float32)).astype(ml_dtypes.bfloat16)
    return hi, lo


def prep_core_inputs(pred_logits, pred_boxes, target_boxes, target_labels, core):
    import ml_dtypes

    s0 = core * NLOC
    pl = np.ascontiguousarray(
        pred_logits[s0 : s0 + NLOC].reshape(NLOC * Q, C), dtype=np.float32
    )
    plp = np.full((NLOC, 384, Q), -30.0, np.float32)
    plp[:, :C, :] = pred_logits[s0 : s0 + NLOC].transpose(0, 2, 1)  # [s, c, q]
    pl_q = np.ascontiguousarray(
        plp.reshape(NLOC, 3, 128, 2, Q // 2).transpose(0, 1, 3, 2, 4)
    ).astype(ml_dtypes.bfloat16)  # [s, cc, qh, ci, l]
    mmrhs = np.zeros((6 * NPAIR, 4, Q), ml_dtypes.bfloat16)
    mmlhs = np.zeros((6 * NPAIR, 4, P2), ml_dtypes.bfloat16)
    for p in range(NPAIR):
        a, b = s0 + 2 * p, s0 + 2 * p + 1
        pa_hi, pa_lo = _bf16_split(pred_boxes[a].T)
        pb_hi, pb_lo = _bf16_split(pred_boxes[b].T)
        ta_hi, ta_lo = _bf16_split(target_boxes[a].T)
        tb_hi, tb_lo = _bf16_split(target_boxes[b].T)
        mmrhs[6 * p + 0] = pa_hi
        mmrhs[6 * p + 1] = pa_lo
        mmrhs[6 * p + 2] = -1.0
        mmrhs[6 * p + 3] = -1.0
        mmrhs[6 * p + 4] = pb_hi
        mmrhs[6 * p + 5] = pb_lo
        mmlhs[6 * p + 0, :, :T] = 1.0
        mmlhs[6 * p + 1, :, :T] = 1.0
        mmlhs[6 * p + 2, :, :T] = ta_hi
        mmlhs[6 * p + 3, :, :T] = ta_lo
        mmlhs[6 * p + 2, :, T:] = tb_hi
        mmlhs[6 * p + 3, :, T:] = tb_lo
        mmlhs[6 * p + 4, :, T:] = 1.0
        mmlhs[6 * p + 5, :, T:] = 1.0
    labels = target_labels[s0 : s0 + NLOC].astype(np.float32)
    m = {
        "logits": pl,
        "logits_q": pl_q,
        "mmrhs": mmrhs,
        "mmlhs": mmlhs,
        "labels": labels,
    }
    m.update(_consts())
    return m


def finalize(loss16_list):
    losses = np.concatenate(
        [np.asarray(l16, np.float32).T.reshape(-1) for l16 in loss16_list]
    )
    return np.float32(losses.mean(dtype=np.float64))


def kernel(pred_logits, pred_boxes, target_boxes, target_labels):
    from concourse.bass_utils import run_bass_kernel_spmd

    pred_logits = np.asarray(pred_logits)
    pred_boxes = np.asarray(pred_boxes)
    target_boxes = np.asarray(target_boxes)
    target_labels = np.asarray(target_labels)

    nc = get_nc()
    in_maps = [
        prep_core_inputs(pred_logits, pred_boxes, target_boxes, target_labels, c)
        for c in range(NCORES)
    ]
    res = run_bass_kernel_spmd(nc, in_maps, core_ids=list(range(NCORES)))
    return finalize([res.results[c]["loss16"] for c in range(NCORES)])

